# revision 2
# baseline (speedup 1.0000x reference)
"""Multi-head attention forward (B=2, S=2048, D=1024, H=16) on 8 Trainium2
NeuronCores, tensor-parallel over heads (2 heads per core).

Per-core program (SPMD, same NEFF, different weight slices):
  - qT/kT/vT projections: qT[d_c, t] = (Wq_c @ x.T)[d_c, t] + bq_c, computed
    from a host-pretransposed xT with fp32r (fast fp32) matmuls.
  - scoresT[k, q] = kT_h.T @ qT_h per (batch, head); exp via ScalarE with the
    1/sqrt(64) folded into the activation scale (no max subtraction needed:
    |scores| < ~3.1 for these inputs).
  - ctxT accumulated over k-tiles with an augmented V (ones column) so the
    softmax denominators fall out of the same matmuls for free.
  - normalize via reciprocal + PE ones-matmul broadcast, then output
    projection against Wo columns; host sums the 8 partial outputs + bo.
"""
import sys
import os

sys.path.insert(0, '/opt/trn_rl_repo')

import numpy as np
import concourse.bass as bass
import concourse.mybir as mybir
import concourse.tile as tile
from concourse import bacc, bass_utils
from concourse.masks import make_identity
import contextlib

f32 = mybir.dt.float32
f32r = mybir.dt.float32r
EXP = mybir.ActivationFunctionType.Exp

B, S, D, H, HD = 2, 2048, 1024, 16, 64
T = B * S              # 4096 tokens
DC = 128               # dims per core (2 heads)
KT = 8                 # feature k-tiles (D / 128)
NCH = 8                # projection chunks of 512 tokens
NKT = 16               # k-token tiles per batch (S / 128)
NQC = 4                # q chunks of 512 per (b, h)


def _build():
    nc = bacc.Bacc("TRN2", target_bir_lowering=False, debug=False)
    xT_d = nc.dram_tensor("xT", [D, T], f32, kind="ExternalInput").ap()
    wqT_d = nc.dram_tensor("wqT", [D, DC], f32, kind="ExternalInput").ap()
    wkT_d = nc.dram_tensor("wkT", [D, DC], f32, kind="ExternalInput").ap()
    wvT_d = nc.dram_tensor("wvT", [D, DC], f32, kind="ExternalInput").ap()
    woT_d = nc.dram_tensor("woT", [DC, D], f32, kind="ExternalInput").ap()
    bq_d = nc.dram_tensor("bq", [DC, 1], f32, kind="ExternalInput").ap()
    bk_d = nc.dram_tensor("bk", [DC, 1], f32, kind="ExternalInput").ap()
    bv_d = nc.dram_tensor("bv", [DC, 1], f32, kind="ExternalInput").ap()
    out_d = nc.dram_tensor("out", [T, D], f32, kind="ExternalOutput").ap()

    xT_ap = xT_d.rearrange("(kt p) t -> p kt t", p=128)

    with tile.TileContext(nc) as tc:
        ctx = contextlib.ExitStack()
        cpool = ctx.enter_context(tc.tile_pool(name="cpool", bufs=1))
        xpool = ctx.enter_context(tc.tile_pool(name="xpool", bufs=2))
        ppool = ctx.enter_context(tc.tile_pool(name="ppool", bufs=6))
        npool = ctx.enter_context(tc.tile_pool(name="npool", bufs=2))
        opool = ctx.enter_context(tc.tile_pool(name="opool", bufs=3))
        pj = ctx.enter_context(tc.tile_pool(name="pj", bufs=2, space="PSUM"))
        vt = ctx.enter_context(tc.tile_pool(name="vt", bufs=1, space="PSUM"))
        sc = ctx.enter_context(tc.tile_pool(name="sc", bufs=3, space="PSUM"))
        cx = ctx.enter_context(tc.tile_pool(name="cx", bufs=1, space="PSUM"))
        bc = ctx.enter_context(tc.tile_pool(name="bc", bufs=1, space="PSUM"))

        # ---- constants / persistent tiles ----
        wqr = cpool.tile([128, KT, DC], f32r, tag="wqr")
        wkr = cpool.tile([128, KT, DC], f32r, tag="wkr")
        wvr = cpool.tile([128, KT, DC], f32r, tag="wvr")
        nc.gpsimd.dma_start(wqr[:], wqT_d.rearrange("(kt p) m -> p kt m", p=128))
        nc.gpsimd.dma_start(wkr[:], wkT_d.rearrange("(kt p) m -> p kt m", p=128))
        nc.gpsimd.dma_start(wvr[:], wvT_d.rearrange("(kt p) m -> p kt m", p=128))
        wor = cpool.tile([128, D], f32r, tag="wor")
        nc.gpsimd.dma_start(wor[:], woT_d[:])
        bq = cpool.tile([DC, 1], f32, tag="bq")
        bk = cpool.tile([DC, 1], f32, tag="bk")
        bv = cpool.tile([DC, 1], f32, tag="bv")
        nc.sync.dma_start(bq[:], bq_d[:])
        nc.sync.dma_start(bk[:], bk_d[:])
        nc.sync.dma_start(bv[:], bv_d[:])

        ident = cpool.tile([128, 128], f32, tag="ident")
        make_identity(nc, ident[:])
        ones = cpool.tile([128, 64], f32, tag="ones")
        nc.vector.memset(ones[:], 1.0)
        onesr = cpool.tile([128, 64], f32r, tag="onesr")
        nc.vector.tensor_copy(onesr[:], ones[:])

        zeros8 = cpool.tile([128, 8, 128], f32, tag="zeros8")
        nc.vector.memset(zeros8[:], 0.0)
        # aug[p, tt*2+h, :]: per k-token-tile per head augmented V operand.
        # h0: v dims at cols 0..63, ones col 64 -> ctx rows 0..63, sums row 64
        # h1: v dims at cols 64..127, ones col 0 -> ctx rows 64..127, sums row 0
        aug = cpool.tile([128, 2 * (B * NKT), 128], f32r, tag="aug")
        for i in range(2 * B * NKT // 8):
            nc.vector.tensor_copy(aug[:, i * 8:(i + 1) * 8, :], zeros8[:])
        for tt in range(B * NKT):
            nc.vector.tensor_copy(aug[:, tt * 2, 64:65], ones[:, 0:1])
            nc.vector.tensor_copy(aug[:, tt * 2 + 1, 0:1], ones[:, 0:1])

        qTr = cpool.tile([128, T], f32r, tag="qTr")
        kTr = cpool.tile([128, T], f32r, tag="kTr")
        vTs = cpool.tile([128, T], f32, tag="vTs")
        ctxT = [cpool.tile([128, S], f32r, tag=f"ctxT{b}", name=f"ctxT{b}")
                for b in range(B)]

        # ---- phase 1: projections + v transposes ----
        for ch in range(NCH):
            csl = slice(ch * 512, (ch + 1) * 512)
            xTr = xpool.tile([128, KT, 512], f32r, tag="xTr")
            nc.gpsimd.dma_start(xTr[:], xT_ap[:, :, csl])
            for wr, b_t, dst in ((wqr, bq, qTr), (wkr, bk, kTr), (wvr, bv, vTs)):
                pp = pj.tile([128, 512], f32, tag="pj")
                for f in range(KT):
                    nc.tensor.matmul(pp[:], wr[:, f], xTr[:, f],
                                     start=(f == 0), stop=(f == KT - 1))
                nc.vector.tensor_scalar_add(dst[:, csl], pp[:], b_t[:])
            vtp = vt.tile([128, 512], f32, tag="vt")
            for j in range(4):
                nc.tensor.matmul(vtp[:, j * 128:(j + 1) * 128],
                                 vTs[:, (ch * 4 + j) * 128:(ch * 4 + j + 1) * 128],
                                 ident[:], is_transpose=True,
                                 start=(j == 0), stop=(j == 3))
            for j in range(4):
                tt = ch * 4 + j
                nc.vector.tensor_copy(aug[:, tt * 2, 0:64],
                                      vtp[:, j * 128:j * 128 + 64])
                nc.vector.tensor_copy(aug[:, tt * 2 + 1, 64:128],
                                      vtp[:, j * 128 + 64:(j + 1) * 128])

        # ---- phase 2: attention ----
        for b in range(B):
            for h in range(2):
                hs = slice(h * 64, (h + 1) * 64)
                for qc in range(NQC):
                    qsl = slice(b * S + qc * 512, b * S + (qc + 1) * 512)
                    osl = slice(qc * 512, (qc + 1) * 512)
                    ctxp = cx.tile([128, 512], f32, tag="cx")
                    for kt in range(NKT):
                        ksl = slice((b * NKT + kt) * 128, (b * NKT + kt + 1) * 128)
                        scp = sc.tile([128, 512], f32, tag="sc")
                        nc.tensor.matmul(scp[:], kTr[hs, ksl], qTr[hs, qsl],
                                         start=True, stop=True)
                        probs = ppool.tile([128, 512], f32r, tag="pb")
                        nc.scalar.activation(probs[:], scp[:], EXP, scale=0.125)
                        nc.tensor.matmul(ctxp[:], aug[:, (b * NKT + kt) * 2 + h, :],
                                         probs[:],
                                         start=(kt == 0), stop=(kt == NKT - 1))
                    if h == 0:
                        # sums at psum row 64; ctx rows 0..63
                        srow = npool.tile([128, 512], f32r, tag="srow")
                        nc.vector.tensor_copy(srow[64:65, :], ctxp[64:65, :])
                        bcp = bc.tile([128, 512], f32, tag="bc")
                        nc.tensor.matmul(bcp[0:64, :], onesr[64:65, 0:64],
                                         srow[64:65, :], start=True, stop=True)
                        bcs = npool.tile([128, 512], f32, tag="bcs")
                        nc.vector.reciprocal_approx_fast(bcs[0:64, :], bcp[0:64, :])
                        nc.vector.tensor_mul(ctxT[b][0:64, osl], ctxp[0:64, :],
                                             bcs[0:64, :])
                    else:
                        # sums at psum row 0; ctx rows 64..127
                        rec = npool.tile([128, 512], f32, tag="rec")
                        nc.vector.reciprocal_approx_fast(rec[0:1, :], ctxp[0:1, :])
                        bcp = bc.tile([128, 512], f32, tag="bc")
                        nc.tensor.matmul(bcp[64:128, :], ones[0:1, 0:64],
                                         rec[0:1, :], start=True, stop=True)
                        cst = npool.tile([128, 512], f32, tag="cst")
                        nc.vector.tensor_copy(cst[64:128, :], ctxp[64:128, :])
                        nc.vector.tensor_mul(ctxT[b][64:128, osl], cst[64:128, :],
                                             bcp[64:128, :])

        # ---- phase 3: output projection (partial over this core's dims) ----
        for b in range(B):
            for tt in range(S // 128):
                ost = opool.tile([128, D], f32, tag="ost")
                for oc in range(2):
                    op = pj.tile([128, 512], f32, tag="pj")
                    nc.tensor.matmul(op[:], ctxT[b][:, tt * 128:(tt + 1) * 128],
                                     wor[:, oc * 512:(oc + 1) * 512],
                                     start=True, stop=True)
                    nc.vector.tensor_copy(ost[:, oc * 512:(oc + 1) * 512], op[:])
                nc.sync.dma_start(
                    out_d[b * S + tt * 128:b * S + (tt + 1) * 128, :], ost[:])
        ctx.close()

    nc.compile()
    return nc


_NC = None


def kernel(inputs, Wq, bq, Wk, bk, Wv, bv, Wo, bo):
    global _NC
    if _NC is None:
        _NC = _build()

    x = np.ascontiguousarray(np.asarray(inputs, dtype=np.float32).reshape(T, D))
    xT = np.ascontiguousarray(x.T)
    Wq = np.asarray(Wq, dtype=np.float32)
    Wk = np.asarray(Wk, dtype=np.float32)
    Wv = np.asarray(Wv, dtype=np.float32)
    Wo = np.asarray(Wo, dtype=np.float32)

    in_maps = []
    for c in range(8):
        sl = slice(c * DC, (c + 1) * DC)
        in_maps.append({
            "xT": xT,
            "wqT": np.ascontiguousarray(Wq[sl].T),
            "wkT": np.ascontiguousarray(Wk[sl].T),
            "wvT": np.ascontiguousarray(Wv[sl].T),
            "woT": np.ascontiguousarray(Wo[:, sl].T),
            "bq": np.ascontiguousarray(np.asarray(bq, np.float32)[sl][:, None]),
            "bk": np.ascontiguousarray(np.asarray(bk, np.float32)[sl][:, None]),
            "bv": np.ascontiguousarray(np.asarray(bv, np.float32)[sl][:, None]),
        })

    res = bass_utils.run_bass_kernel_spmd(_NC, in_maps, core_ids=list(range(8)))
    out = res.results[0]["out"].astype(np.float32)
    for r in res.results[1:]:
        out += r["out"]
    out += np.asarray(bo, dtype=np.float32)[None, :]
    return out.reshape(B, S, D)


# revision 5
# speedup vs baseline: 1.2718x; 1.2718x over previous
"""Multi-head attention forward (B=2, S=2048, D=1024, H=16) on 8 Trainium2
NeuronCores, tensor-parallel over heads (2 heads per core).

Per-core program (SPMD, same NEFF, different weight slices):
  - qT/kT/vT projections: qT[d_c, t] = (Wq_c @ x.T)[d_c, t] + bq_c, computed
    from a host-pretransposed xT with fp32r (fast fp32) matmuls.
  - scoresT[k, q] = kT_h.T @ qT_h per (batch, head); exp via ScalarE with the
    1/sqrt(64) folded into the activation scale (no max subtraction needed:
    |scores| < ~3.1 for these inputs).
  - ctxT accumulated over k-tiles with an augmented V (ones column) so the
    softmax denominators fall out of the same matmuls for free.
  - normalize via reciprocal + PE ones-matmul broadcast, then output
    projection against Wo columns; host sums the 8 partial outputs + bo.
"""
import sys
import os

sys.path.insert(0, '/opt/trn_rl_repo')

import numpy as np
import concourse.bass as bass
import concourse.mybir as mybir
import concourse.tile as tile
from concourse import bacc, bass_utils
from concourse.masks import make_identity
import contextlib

f32 = mybir.dt.float32
f32r = mybir.dt.float32r
EXP = mybir.ActivationFunctionType.Exp

B, S, D, H, HD = 2, 2048, 1024, 16, 64
T = B * S              # 4096 tokens
DC = 128               # dims per core (2 heads)
KT = 8                 # feature k-tiles (D / 128)
NCH = 8                # projection chunks of 512 tokens
NKT = 16               # k-token tiles per batch (S / 128)
NQC = 4                # q chunks of 512 per (b, h)


def _build():
    nc = bacc.Bacc("TRN2", target_bir_lowering=False, debug=False)
    xT_d = nc.dram_tensor("xT", [D, T], f32, kind="ExternalInput").ap()
    wqT_d = nc.dram_tensor("wqT", [D, DC], f32, kind="ExternalInput").ap()
    wkT_d = nc.dram_tensor("wkT", [D, DC], f32, kind="ExternalInput").ap()
    wvT_d = nc.dram_tensor("wvT", [D, DC], f32, kind="ExternalInput").ap()
    woT_d = nc.dram_tensor("woT", [DC, D], f32, kind="ExternalInput").ap()
    bq_d = nc.dram_tensor("bq", [DC, 1], f32, kind="ExternalInput").ap()
    bk_d = nc.dram_tensor("bk", [DC, 1], f32, kind="ExternalInput").ap()
    bv_d = nc.dram_tensor("bv", [DC, 1], f32, kind="ExternalInput").ap()
    out_d = nc.dram_tensor("out", [T, D], f32, kind="ExternalOutput").ap()

    xT_ap = xT_d.rearrange("(kt p) t -> p kt t", p=128)

    with tile.TileContext(nc) as tc:
        ctx = contextlib.ExitStack()
        cpool = ctx.enter_context(tc.tile_pool(name="cpool", bufs=1))
        xpool = ctx.enter_context(tc.tile_pool(name="xpool", bufs=2))
        ppool = ctx.enter_context(tc.tile_pool(name="ppool", bufs=6))
        npool = ctx.enter_context(tc.tile_pool(name="npool", bufs=2))
        opool = ctx.enter_context(tc.tile_pool(name="opool", bufs=3))
        pj = ctx.enter_context(tc.tile_pool(name="pj", bufs=1, space="PSUM"))
        sc = ctx.enter_context(tc.tile_pool(name="sc", bufs=2, space="PSUM"))
        cx = ctx.enter_context(tc.tile_pool(name="cx", bufs=3, space="PSUM"))

        # ---- constants / persistent tiles ----
        wqr = cpool.tile([128, KT, DC], f32r, tag="wqr")
        wkr = cpool.tile([128, KT, DC], f32r, tag="wkr")
        wvr = cpool.tile([128, KT, DC], f32r, tag="wvr")
        nc.gpsimd.dma_start(wqr[:], wqT_d.rearrange("(kt p) m -> p kt m", p=128))
        nc.gpsimd.dma_start(wkr[:], wkT_d.rearrange("(kt p) m -> p kt m", p=128))
        nc.gpsimd.dma_start(wvr[:], wvT_d.rearrange("(kt p) m -> p kt m", p=128))
        wor = cpool.tile([128, D], f32r, tag="wor")
        nc.gpsimd.dma_start(wor[:], woT_d[:])
        bq = cpool.tile([DC, 1], f32, tag="bq")
        bk = cpool.tile([DC, 1], f32, tag="bk")
        bv = cpool.tile([DC, 1], f32, tag="bv")
        nc.sync.dma_start(bq[:], bq_d[:])
        nc.sync.dma_start(bk[:], bk_d[:])
        nc.sync.dma_start(bv[:], bv_d[:])

        ident = cpool.tile([128, 128], f32, tag="ident")
        make_identity(nc, ident[:])
        ones = cpool.tile([128, 64], f32, tag="ones")
        nc.vector.memset(ones[:], 1.0)
        onesr = cpool.tile([128, 64], f32r, tag="onesr")
        nc.vector.tensor_copy(onesr[:], ones[:])

        zeros8 = cpool.tile([128, 8, 128], f32, tag="zeros8")
        nc.vector.memset(zeros8[:], 0.0)
        # aug[p, tt*2+h, :]: per k-token-tile per head augmented V operand.
        # h0: v dims at cols 0..63, ones col 64 -> ctx rows 0..63, sums row 64
        # h1: v dims at cols 64..127, ones col 0 -> ctx rows 64..127, sums row 0
        aug = cpool.tile([128, 2 * (B * NKT), 128], f32r, tag="aug")
        for i in range(2 * B * NKT // 8):
            nc.vector.tensor_copy(aug[:, i * 8:(i + 1) * 8, :], zeros8[:])
        for tt in range(B * NKT):
            nc.vector.tensor_copy(aug[:, tt * 2, 64:65], ones[:, 0:1])
            nc.vector.tensor_copy(aug[:, tt * 2 + 1, 0:1], ones[:, 0:1])

        qTr = cpool.tile([128, T], f32r, tag="qTr")
        kTr = cpool.tile([128, T], f32r, tag="kTr")
        vTs = cpool.tile([128, T], f32, tag="vTs")
        ctxT = [cpool.tile([128, S], f32r, tag=f"ctxT{b}", name=f"ctxT{b}")
                for b in range(B)]

        # ---- phase 1: projections + v transposes ----
        for ch in range(NCH):
            csl = slice(ch * 512, (ch + 1) * 512)
            xTr = xpool.tile([128, KT, 512], f32r, tag="xTr")
            nc.gpsimd.dma_start(xTr[:], xT_ap[:, :, csl])
            for wr, b_t, dst in ((wqr, bq, qTr), (wkr, bk, kTr), (wvr, bv, vTs)):
                pp = pj.tile([128, 512], f32, tag="pj")
                for f in range(KT):
                    nc.tensor.matmul(pp[:], wr[:, f], xTr[:, f],
                                     start=(f == 0), stop=(f == KT - 1))
                nc.vector.tensor_scalar_add(dst[:, csl], pp[:], b_t[:])
            vtp = pj.tile([128, 512], f32, tag="pj")
            for j in range(4):
                nc.tensor.matmul(vtp[:, j * 128:(j + 1) * 128],
                                 vTs[:, (ch * 4 + j) * 128:(ch * 4 + j + 1) * 128],
                                 ident[:], is_transpose=True,
                                 start=(j == 0), stop=(j == 3))
            for j in range(4):
                tt = ch * 4 + j
                nc.vector.tensor_copy(aug[:, tt * 2, 0:64],
                                      vtp[:, j * 128:j * 128 + 64])
                nc.vector.tensor_copy(aug[:, tt * 2 + 1, 64:128],
                                      vtp[:, j * 128 + 64:(j + 1) * 128])

        # ---- phase 2/3: attention + output projection, interleaved per batch ----
        def attention(b):
            for h in range(2):
                hs = slice(h * 64, (h + 1) * 64)
                for qc in range(NQC):
                    qsl = slice(b * S + qc * 512, b * S + (qc + 1) * 512)
                    osl = slice(qc * 512, (qc + 1) * 512)
                    ctxp = cx.tile([128, 512], f32, tag="cx", name="ctxp")
                    # process k-tiles in pairs: two kt's scoresT share one
                    # 2-bank psum tile so a single 1024-wide exp covers both
                    for kp in range(NKT // 2):
                        scp = sc.tile([128, 1024], f32, tag="sc", name="scp")
                        probs = ppool.tile([128, 1024], f32r, tag="pb",
                                           name="probs")
                        for j in range(2):
                            kt = kp * 2 + j
                            ksl = slice((b * NKT + kt) * 128,
                                        (b * NKT + kt + 1) * 128)
                            nc.tensor.matmul(scp[:, j * 512:(j + 1) * 512],
                                             kTr[hs, ksl], qTr[hs, qsl],
                                             start=True, stop=True)
                        nc.scalar.activation(probs[:], scp[:], EXP, scale=0.125)
                        for j in range(2):
                            kt = kp * 2 + j
                            nc.tensor.matmul(
                                ctxp[:], aug[:, (b * NKT + kt) * 2 + h, :],
                                probs[:, j * 512:(j + 1) * 512],
                                start=(kt == 0), stop=(kt == NKT - 1))
                    if h == 0:
                        # sums at psum row 64; ctx rows 0..63
                        srow = npool.tile([128, 512], f32r, tag="srow")
                        nc.vector.tensor_copy(srow[64:65, :], ctxp[64:65, :])
                        bcp = cx.tile([128, 512], f32, tag="cx", name="bcp")
                        nc.tensor.matmul(bcp[0:64, :], onesr[64:65, 0:64],
                                         srow[64:65, :], start=True, stop=True)
                        bcs = npool.tile([128, 512], f32, tag="bcs")
                        nc.vector.reciprocal_approx_fast(bcs[0:64, :], bcp[0:64, :])
                        nc.vector.tensor_mul(ctxT[b][0:64, osl], ctxp[0:64, :],
                                             bcs[0:64, :])
                    else:
                        # sums at psum row 0; ctx rows 64..127
                        rec = npool.tile([128, 512], f32, tag="rec")
                        nc.vector.reciprocal_approx_fast(rec[0:1, :], ctxp[0:1, :])
                        bcp = cx.tile([128, 512], f32, tag="cx", name="bcp")
                        nc.tensor.matmul(bcp[64:128, :], ones[0:1, 0:64],
                                         rec[0:1, :], start=True, stop=True)
                        cst = npool.tile([128, 512], f32, tag="cst")
                        nc.vector.tensor_copy(cst[64:128, :], ctxp[64:128, :])
                        nc.vector.tensor_mul(ctxT[b][64:128, osl], cst[64:128, :],
                                             bcp[64:128, :])

        def out_proj(b):
            for tt in range(S // 128):
                ost = opool.tile([128, D], f32, tag="ost", name="ost")
                for oc in range(2):
                    op = pj.tile([128, 512], f32, tag="pj", name="op")
                    nc.tensor.matmul(op[:], ctxT[b][:, tt * 128:(tt + 1) * 128],
                                     wor[:, oc * 512:(oc + 1) * 512],
                                     start=True, stop=True)
                    nc.vector.tensor_copy(ost[:, oc * 512:(oc + 1) * 512], op[:])
                nc.sync.dma_start(
                    out_d[b * S + tt * 128:b * S + (tt + 1) * 128, :], ost[:])

        attention(0)
        out_proj(0)      # PE filler work while attention(1) waits on exp
        attention(1)
        out_proj(1)
        ctx.close()

    nc.compile()
    return nc


_NC = None


def kernel(inputs, Wq, bq, Wk, bk, Wv, bv, Wo, bo):
    global _NC
    if _NC is None:
        _NC = _build()

    x = np.ascontiguousarray(np.asarray(inputs, dtype=np.float32).reshape(T, D))
    xT = np.ascontiguousarray(x.T)
    Wq = np.asarray(Wq, dtype=np.float32)
    Wk = np.asarray(Wk, dtype=np.float32)
    Wv = np.asarray(Wv, dtype=np.float32)
    Wo = np.asarray(Wo, dtype=np.float32)

    in_maps = []
    for c in range(8):
        sl = slice(c * DC, (c + 1) * DC)
        in_maps.append({
            "xT": xT,
            "wqT": np.ascontiguousarray(Wq[sl].T),
            "wkT": np.ascontiguousarray(Wk[sl].T),
            "wvT": np.ascontiguousarray(Wv[sl].T),
            "woT": np.ascontiguousarray(Wo[:, sl].T),
            "bq": np.ascontiguousarray(np.asarray(bq, np.float32)[sl][:, None]),
            "bk": np.ascontiguousarray(np.asarray(bk, np.float32)[sl][:, None]),
            "bv": np.ascontiguousarray(np.asarray(bv, np.float32)[sl][:, None]),
        })

    res = bass_utils.run_bass_kernel_spmd(_NC, in_maps, core_ids=list(range(8)))
    out = res.results[0]["out"].astype(np.float32)
    for r in res.results[1:]:
        out += r["out"]
    out += np.asarray(bo, dtype=np.float32)[None, :]
    return out.reshape(B, S, D)


# revision 7
# speedup vs baseline: 1.3456x; 1.0580x over previous
"""Multi-head attention forward (B=2, S=2048, D=1024, H=16) on 8 Trainium2
NeuronCores, tensor-parallel over heads (2 heads per core).

Per-core program (SPMD, same NEFF, different weight slices):
  - qT/kT/vT projections: qT[d_c, t] = (Wq_c @ x.T)[d_c, t] + bq_c, computed
    from a host-pretransposed xT with fp32r (fast fp32) matmuls.
  - scoresT[k, q] = kT_h.T @ qT_h per (batch, head); exp via ScalarE with the
    1/sqrt(64) folded into the activation scale (no max subtraction needed:
    |scores| < ~3.1 for these inputs).
  - ctxT accumulated over k-tiles with an augmented V (ones column) so the
    softmax denominators fall out of the same matmuls for free.
  - normalize via reciprocal + PE ones-matmul broadcast, then output
    projection against Wo columns; host sums the 8 partial outputs + bo.
"""
import sys
import os

sys.path.insert(0, '/opt/trn_rl_repo')

import numpy as np
import concourse.bass as bass
import concourse.mybir as mybir
import concourse.tile as tile
from concourse import bacc, bass_utils
from concourse.masks import make_identity
import contextlib

f32 = mybir.dt.float32
f32r = mybir.dt.float32r
EXP = mybir.ActivationFunctionType.Exp

B, S, D, H, HD = 2, 2048, 1024, 16, 64
T = B * S              # 4096 tokens
DC = 128               # dims per core (2 heads)
KT = 8                 # feature k-tiles (D / 128)
NCH = 8                # projection chunks of 512 tokens
NKT = 16               # k-token tiles per batch (S / 128)
NQC = 4                # q chunks of 512 per (b, h)


def _build():
    nc = bacc.Bacc("TRN2", target_bir_lowering=False, debug=False)
    xT_d = nc.dram_tensor("xT", [D, T], f32, kind="ExternalInput").ap()
    wqT_d = nc.dram_tensor("wqT", [D, DC], f32, kind="ExternalInput").ap()
    wkT_d = nc.dram_tensor("wkT", [D, DC], f32, kind="ExternalInput").ap()
    wvT_d = nc.dram_tensor("wvT", [D, DC], f32, kind="ExternalInput").ap()
    woT_d = nc.dram_tensor("woT", [DC, D], f32, kind="ExternalInput").ap()
    bq_d = nc.dram_tensor("bq", [DC, 1], f32, kind="ExternalInput").ap()
    bk_d = nc.dram_tensor("bk", [DC, 1], f32, kind="ExternalInput").ap()
    bv_d = nc.dram_tensor("bv", [DC, 1], f32, kind="ExternalInput").ap()
    out_d = nc.dram_tensor("out", [T, D], f32, kind="ExternalOutput").ap()

    xT_ap = xT_d.rearrange("(kt p) t -> p kt t", p=128)

    with tile.TileContext(nc) as tc:
        ctx = contextlib.ExitStack()
        cpool = ctx.enter_context(tc.tile_pool(name="cpool", bufs=1))
        xpool = ctx.enter_context(tc.tile_pool(name="xpool", bufs=2))
        ppool = ctx.enter_context(tc.tile_pool(name="ppool", bufs=6))
        npool = ctx.enter_context(tc.tile_pool(name="npool", bufs=2))
        opool = ctx.enter_context(tc.tile_pool(name="opool", bufs=3))
        pj = ctx.enter_context(tc.tile_pool(name="pj", bufs=2, space="PSUM"))
        sc = ctx.enter_context(tc.tile_pool(name="sc", bufs=2, space="PSUM"))
        cx = ctx.enter_context(tc.tile_pool(name="cx", bufs=2, space="PSUM"))

        # ---- constants / persistent tiles ----
        wqr = cpool.tile([128, KT, DC], f32r, tag="wqr")
        wkr = cpool.tile([128, KT, DC], f32r, tag="wkr")
        wvr = cpool.tile([128, KT, DC], f32r, tag="wvr")
        nc.gpsimd.dma_start(wqr[:], wqT_d.rearrange("(kt p) m -> p kt m", p=128))
        nc.gpsimd.dma_start(wkr[:], wkT_d.rearrange("(kt p) m -> p kt m", p=128))
        nc.gpsimd.dma_start(wvr[:], wvT_d.rearrange("(kt p) m -> p kt m", p=128))
        wor = cpool.tile([128, D], f32r, tag="wor")
        nc.gpsimd.dma_start(wor[:], woT_d[:])
        bq = cpool.tile([DC, 1], f32, tag="bq")
        bk = cpool.tile([DC, 1], f32, tag="bk")
        bv = cpool.tile([DC, 1], f32, tag="bv")
        nc.sync.dma_start(bq[:], bq_d[:])
        nc.sync.dma_start(bk[:], bk_d[:])
        nc.sync.dma_start(bv[:], bv_d[:])

        ident = cpool.tile([128, 128], f32, tag="ident")
        make_identity(nc, ident[:])
        ones = cpool.tile([128, 64], f32, tag="ones")
        nc.vector.memset(ones[:], 1.0)
        onesr = cpool.tile([128, 64], f32r, tag="onesr")
        nc.vector.tensor_copy(onesr[:], ones[:])

        zeros8 = cpool.tile([128, 8, 128], f32, tag="zeros8")
        nc.vector.memset(zeros8[:], 0.0)
        # aug[p, tt*2+h, :]: per k-token-tile per head augmented V operand.
        # h0: v dims at cols 0..63, ones col 64 -> ctx rows 0..63, sums row 64
        # h1: v dims at cols 64..127, ones col 0 -> ctx rows 64..127, sums row 0
        aug = cpool.tile([128, 2 * (B * NKT), 128], f32r, tag="aug")
        for i in range(2 * B * NKT // 8):
            nc.vector.tensor_copy(aug[:, i * 8:(i + 1) * 8, :], zeros8[:])
        for tt in range(B * NKT):
            nc.vector.tensor_copy(aug[:, tt * 2, 64:65], ones[:, 0:1])
            nc.vector.tensor_copy(aug[:, tt * 2 + 1, 0:1], ones[:, 0:1])

        qTr = cpool.tile([128, T], f32r, tag="qTr")
        kTr = cpool.tile([128, T], f32r, tag="kTr")
        vTs = cpool.tile([128, T], f32, tag="vTs")
        ctxT = [cpool.tile([128, S], f32r, tag=f"ctxT{b}", name=f"ctxT{b}")
                for b in range(B)]

        # ---- phase 1: projections + v transposes ----
        for ch in range(NCH):
            csl = slice(ch * 512, (ch + 1) * 512)
            xTr = xpool.tile([128, KT, 512], f32r, tag="xTr")
            if ch == 0:
                # split the first chunk's load per feature tile so the first
                # projection matmuls can start as soon as f=0 lands
                for f in range(KT):
                    nc.gpsimd.dma_start(xTr[:, f], xT_ap[:, f, csl])
            else:
                nc.gpsimd.dma_start(xTr[:], xT_ap[:, :, csl])
            for wr, b_t, dst in ((wqr, bq, qTr), (wkr, bk, kTr), (wvr, bv, vTs)):
                pp = pj.tile([128, 512], f32, tag="pj")
                for f in range(KT):
                    nc.tensor.matmul(pp[:], wr[:, f], xTr[:, f],
                                     start=(f == 0), stop=(f == KT - 1))
                nc.vector.tensor_scalar_add(dst[:, csl], pp[:], b_t[:])
            vtp = pj.tile([128, 512], f32, tag="pj")
            for j in range(4):
                nc.tensor.matmul(vtp[:, j * 128:(j + 1) * 128],
                                 vTs[:, (ch * 4 + j) * 128:(ch * 4 + j + 1) * 128],
                                 ident[:], is_transpose=True,
                                 start=(j == 0), stop=(j == 3))
            for j in range(4):
                tt = ch * 4 + j
                nc.vector.tensor_copy(aug[:, tt * 2, 0:64],
                                      vtp[:, j * 128:j * 128 + 64])
                nc.vector.tensor_copy(aug[:, tt * 2 + 1, 64:128],
                                      vtp[:, j * 128 + 64:(j + 1) * 128])

        # ---- phase 2/3: attention + output projection, interleaved per batch ----
        def attention(b):
            for h in range(2):
                hs = slice(h * 64, (h + 1) * 64)
                for qc in range(NQC):
                    qsl = slice(b * S + qc * 512, b * S + (qc + 1) * 512)
                    osl = slice(qc * 512, (qc + 1) * 512)
                    ctxp = cx.tile([128, 512], f32, tag="cx", name="ctxp")
                    # process k-tiles in pairs: two kt's scoresT share one
                    # 2-bank psum tile so a single 1024-wide exp covers both
                    for kp in range(NKT // 2):
                        scp = sc.tile([128, 1024], f32, tag="sc", name="scp")
                        probs = ppool.tile([128, 1024], f32r, tag="pb",
                                           name="probs")
                        for j in range(2):
                            kt = kp * 2 + j
                            ksl = slice((b * NKT + kt) * 128,
                                        (b * NKT + kt + 1) * 128)
                            nc.tensor.matmul(scp[:, j * 512:(j + 1) * 512],
                                             kTr[hs, ksl], qTr[hs, qsl],
                                             start=True, stop=True)
                        nc.scalar.activation(probs[:], scp[:], EXP, scale=0.125)
                        for j in range(2):
                            kt = kp * 2 + j
                            nc.tensor.matmul(
                                ctxp[:], aug[:, (b * NKT + kt) * 2 + h, :],
                                probs[:, j * 512:(j + 1) * 512],
                                start=(kt == 0), stop=(kt == NKT - 1))
                    if h == 0:
                        # sums at psum row 64; ctx rows 0..63
                        srow = npool.tile([128, 512], f32r, tag="srow")
                        nc.vector.tensor_copy(srow[64:65, :], ctxp[64:65, :])
                        bcp = cx.tile([128, 512], f32, tag="cx", name="bcp")
                        nc.tensor.matmul(bcp[0:64, :], onesr[64:65, 0:64],
                                         srow[64:65, :], start=True, stop=True)
                        bcs = npool.tile([128, 512], f32, tag="bcs")
                        nc.vector.reciprocal_approx_fast(bcs[0:64, :], bcp[0:64, :])
                        nc.vector.tensor_mul(ctxT[b][0:64, osl], ctxp[0:64, :],
                                             bcs[0:64, :])
                    else:
                        # sums at psum row 0; ctx rows 64..127
                        rec = npool.tile([128, 512], f32, tag="rec")
                        nc.vector.reciprocal_approx_fast(rec[0:1, :], ctxp[0:1, :])
                        bcp = cx.tile([128, 512], f32, tag="cx", name="bcp")
                        nc.tensor.matmul(bcp[64:128, :], ones[0:1, 0:64],
                                         rec[0:1, :], start=True, stop=True)
                        cst = npool.tile([128, 512], f32, tag="cst")
                        nc.vector.tensor_copy(cst[64:128, :], ctxp[64:128, :])
                        nc.vector.tensor_mul(ctxT[b][64:128, osl], cst[64:128, :],
                                             bcp[64:128, :])

        def out_proj(b):
            for tt in range(S // 128):
                ost = opool.tile([128, D], f32, tag="ost", name="ost")
                for oc in range(2):
                    op = pj.tile([128, 512], f32, tag="pj", name="op")
                    nc.tensor.matmul(op[:], ctxT[b][:, tt * 128:(tt + 1) * 128],
                                     wor[:, oc * 512:(oc + 1) * 512],
                                     start=True, stop=True)
                    nc.vector.tensor_copy(ost[:, oc * 512:(oc + 1) * 512], op[:])
                nc.sync.dma_start(
                    out_d[b * S + tt * 128:b * S + (tt + 1) * 128, :], ost[:])

        attention(0)
        out_proj(0)      # PE filler work while attention(1) waits on exp
        attention(1)
        out_proj(1)
        ctx.close()

    nc.compile()
    return nc


_NC = None


def kernel(inputs, Wq, bq, Wk, bk, Wv, bv, Wo, bo):
    global _NC
    if _NC is None:
        _NC = _build()

    x = np.ascontiguousarray(np.asarray(inputs, dtype=np.float32).reshape(T, D))
    xT = np.ascontiguousarray(x.T)
    Wq = np.asarray(Wq, dtype=np.float32)
    Wk = np.asarray(Wk, dtype=np.float32)
    Wv = np.asarray(Wv, dtype=np.float32)
    Wo = np.asarray(Wo, dtype=np.float32)

    in_maps = []
    for c in range(8):
        sl = slice(c * DC, (c + 1) * DC)
        in_maps.append({
            "xT": xT,
            "wqT": np.ascontiguousarray(Wq[sl].T),
            "wkT": np.ascontiguousarray(Wk[sl].T),
            "wvT": np.ascontiguousarray(Wv[sl].T),
            "woT": np.ascontiguousarray(Wo[:, sl].T),
            "bq": np.ascontiguousarray(np.asarray(bq, np.float32)[sl][:, None]),
            "bk": np.ascontiguousarray(np.asarray(bk, np.float32)[sl][:, None]),
            "bv": np.ascontiguousarray(np.asarray(bv, np.float32)[sl][:, None]),
        })

    res = bass_utils.run_bass_kernel_spmd(_NC, in_maps, core_ids=list(range(8)))
    out = res.results[0]["out"].astype(np.float32)
    for r in res.results[1:]:
        out += r["out"]
    out += np.asarray(bo, dtype=np.float32)[None, :]
    return out.reshape(B, S, D)
